# revision 10
# baseline (speedup 1.0000x reference)
"""Trainium2 kernel for AdaptiveAttention (QKV projection + causal
sliding-window attention, span=128) on 8 NeuronCores.

Sharding: sequence-parallel with a 1-block halo — 8 shards of
(batch b, sequence half h): each core owns 2048 query tokens and receives
128 halo tokens of x so it can compute the previous block's K/V locally.
No collectives are needed (window attention is local).

Per-core layout strategy (all matmuls in float32r, full PE speed at N>=256):
  - x is passed pre-transposed (d-major) so Q^T/K^T projections and the
    V projection need no on-chip transposes:
      Q^T[e,t] = sum_d WqT[d,e] * xT[d,t]   (lhsT=WqT tile, rhs=xT)
      V[t,e]   = sum_d xT[d,t]  * WvT[d,e]  (lhsT=xT tile,  rhs=WvT)
  - scores are computed transposed, sT[k,q] = sum_e KT[e,k]*QT[e,q],
    masked + scaled on DVE, exponentiated on ACT (no max subtraction:
    scores are O(+-8) so exp is safe in fp32), then
      o[q,e] = sum_k wT[k,q] * V[k,e]
    with the softmax denominator obtained from an extra ones-column
    matmul into the same PSUM tile; final normalize is a per-partition
    scalar multiply fused with the PSUM->SBUF copy.
"""

import sys
import types

if "/opt/trn_rl_repo" not in sys.path:
    sys.path.insert(0, "/opt/trn_rl_repo")

import numpy as np
from contextlib import ExitStack

import concourse.bass as bass
import concourse.mybir as mybir
import concourse.tile as tile
from concourse.bass_utils import run_bass_kernel_spmd
from concourse.vector_clock import ScopedClock

# ---------------------------------------------------------------------------
# Problem constants (hardcoded per spec)
B, T, D = 4, 4096, 1024
SPAN = 128
NCORES = 8
TOKQ = T // 2           # 2048 query tokens per core
HALO = SPAN             # 128
CH = 256                # query chunk size (2 blocks = 1 block-pair)
NCH = TOKQ // CH        # 8 chunks
KVW = CH + HALO         # 384 KV tokens visible per chunk
DT = D // 128           # 8 d-tiles
NEG = -1.0e9
SCALE = 1.0 / 32.0      # 1/sqrt(D)

F32 = mybir.dt.float32
F32R = mybir.dt.float32r

# ---------------------------------------------------------------------------
# Walrus in this toolchain caps semaphore waits per instruction; Tile's
# kernel-tail Drain can exceed it. Chunk excess waits onto extra drains.
_MAX_WAITS = 1


def _patched_drain_and_barrier(self, tick_clock, wait_clock):
    nc = self.nc
    drain_inst = nc.sync.drain()
    wait_clock.add_sem_waits(
        drain_inst.ins, ScopedClock({None: tick_clock.global_clock})
    )
    si = drain_inst.ins.sync_info
    if si is not None and len(si.on_wait) > _MAX_WAITS:
        waits = list(si.on_wait)
        si.on_wait[:] = waits[:_MAX_WAITS]
        rest = waits[_MAX_WAITS:]
        while rest:
            extra = nc.sync.drain(fusable=False)
            extra.ins.sync_info = mybir.SyncInfo(
                on_wait=rest[:_MAX_WAITS], on_update=[]
            )
            rest = rest[_MAX_WAITS:]
    nc.all_engine_barrier()
    assert self.sems is not None
    popped = nc._tile_sem_poison_stack.pop()
    assert popped is self._sem_poison
    nc.clear_and_free_semaphores(list(self.sems.allocated().values()))
    nc.all_engine_barrier()


def _install_drain_patch():
    if getattr(tile.TileContext, "_drain_patch_installed", False):
        return
    tile.TileContext._drain_and_barrier = _patched_drain_and_barrier
    tile.TileContext._drain_patch_installed = True


def _split_multi_waits(nc, max_waits=_MAX_WAITS):
    """Walrus here supports one semaphore wait per instruction; hoist excess
    waits onto same-engine NoOps inserted immediately before."""
    for fn in nc.m.functions:
        for bb in fn.blocks:
            insts = bb.instructions
            out = []
            changed = False
            for inst in insts:
                si = getattr(inst, "sync_info", None)
                waits = list(si.on_wait) if si is not None else []
                if len(waits) > max_waits:
                    changed = True
                    for w in waits[:-max_waits]:
                        out.append(mybir.InstNoOp(
                            name=nc.get_next_instruction_name(),
                            sync_info=mybir.SyncInfo(on_wait=[w], on_update=[]),
                            bass_nofuse=True,
                            engine=inst.engine,
                        ))
                    si.on_wait[:] = waits[-max_waits:]
                out.append(inst)
            if changed:
                bb.instructions = out


# ---------------------------------------------------------------------------
def _build_graph():
    """Build the per-core Bass graph (SPMD: identical on all 8 cores)."""
    _install_drain_patch()
    nc = bass.Bass()

    # DRAM parameters. Matmul operands are declared float32r (same bits as
    # f32; the PE rounds to its reduced internal format).
    xT = nc.declare_dram_parameter("xT", [D, HALO + TOKQ], F32R, isOutput=False)
    wqT = nc.declare_dram_parameter("wqT", [D, D], F32R, isOutput=False)
    wkT = nc.declare_dram_parameter("wkT", [D, D], F32R, isOutput=False)
    wvT = nc.declare_dram_parameter("wvT", [D, D], F32R, isOutput=False)
    bq8 = nc.declare_dram_parameter("bq8", [128, DT], F32, isOutput=False)
    bk8 = nc.declare_dram_parameter("bk8", [128, DT], F32, isOutput=False)
    bvb = nc.declare_dram_parameter("bvb", [128, D], F32, isOutput=False)
    maskc = nc.declare_dram_parameter("maskc", [128, 3 * CH], F32, isOutput=False)
    maskf = nc.declare_dram_parameter("maskf", [128, 3 * CH], F32, isOutput=False)
    onesp = nc.declare_dram_parameter("onesp", [128, 8], F32R, isOutput=False)
    out = nc.declare_dram_parameter("out", [TOKQ, D], F32, isOutput=True)

    with ExitStack() as ctx:
        tc = ctx.enter_context(tile.TileContext(nc))
        consts = ctx.enter_context(tc.tile_pool(name="consts", bufs=1))
        persist = ctx.enter_context(tc.tile_pool(name="persist", bufs=1))
        xc0p = ctx.enter_context(tc.tile_pool(name="xc0p", bufs=1))
        xcp = ctx.enter_context(tc.tile_pool(name="xcp", bufs=2))
        work = ctx.enter_context(tc.tile_pool(name="work", bufs=1))
        outp = ctx.enter_context(tc.tile_pool(name="outp", bufs=2))
        psp = ctx.enter_context(tc.tile_pool(name="psp", bufs=2, space="PSUM"))
        pss = ctx.enter_context(tc.tile_pool(name="pss", bufs=1, space="PSUM"))
        pso = ctx.enter_context(tc.tile_pool(name="pso", bufs=1, space="PSUM"))

        # --- resident weights + constants -------------------------------
        # DMA order matters: the K projection of chunk 0 needs xc0 + Wk, so
        # issue those first to start the PE as early as possible; Wq next
        # (Q proj), Wv last (V proj runs after K proj in chunk 0).
        xc0 = xc0p.tile([128, DT, KVW], F32R, tag="xc0")
        wk_sb = []
        for d in range(DT):
            nc.sync.dma_start(
                out=xc0[:, d, :], in_=xT[d * 128:(d + 1) * 128, 0:KVW]
            )
            wk = consts.tile([128, D], F32R, tag=f"wk{d}")
            nc.sync.dma_start(out=wk, in_=wkT[d * 128:(d + 1) * 128, :])
            wk_sb.append(wk)
        wq_sb, wv_sb = [], []
        for d in range(DT):
            wv = consts.tile([128, D], F32R, tag=f"wv{d}")
            nc.sync.dma_start(out=wv, in_=wvT[d * 128:(d + 1) * 128, :])
            wv_sb.append(wv)
        for d in range(DT):
            wq = consts.tile([128, D], F32R, tag=f"wq{d}")
            nc.sync.dma_start(out=wq, in_=wqT[d * 128:(d + 1) * 128, :])
            wq_sb.append(wq)

        bq_sb = consts.tile([128, DT], F32, tag="bq")
        nc.sync.dma_start(out=bq_sb, in_=bq8[:, :])
        bk_sb = consts.tile([128, DT], F32, tag="bk")
        nc.sync.dma_start(out=bk_sb, in_=bk8[:, :])
        bv_sb = consts.tile([128, D], F32, tag="bv")
        nc.sync.dma_start(out=bv_sb, in_=bvb[:, :])
        mask_sb = consts.tile([128, 3 * CH], F32, tag="mask")
        nc.sync.dma_start(out=mask_sb, in_=maskc[:, :])
        maskf_sb = consts.tile([128, 3 * CH], F32, tag="maskf")
        nc.sync.dma_start(out=maskf_sb, in_=maskf[:, :])
        ones_sb = consts.tile([128, 8], F32R, tag="ones")
        nc.sync.dma_start(out=ones_sb, in_=onesp[:, :])

        # --- persistent per-chunk state ---------------------------------
        # KT: K^T, e-tile-major [128, e_tile, kv_col], kv_col in [0, 384):
        #   kv token (chunk-local) = chunk_start - 128 + kv_col
        # V: token-tile-major [128, tok_tile(3), e]
        # QT: Q^T [128, e_tile, q_col], q_col in [0, 256)
        KT = persist.tile([128, DT, KVW], F32R, tag="KT")
        V = persist.tile([128, KVW // 128, D], F32R, tag="V")
        QT = persist.tile([128, DT, CH], F32R, tag="QT")

        for c in range(NCH):
            # ---- x chunk DMA (d-major) ---------------------------------
            if c == 0:
                # xc0 (incl. halo: xT cols [0, 384)) was DMA'd up front
                xc = xc0
                own0 = HALO          # xc col of first own token
                kv_t0 = 0            # first KV token-tile to project
            else:
                xc = xcp.tile([128, DT, CH], F32R, tag="xc")
                lo = HALO + c * CH
                for d in range(DT):
                    nc.sync.dma_start(
                        out=xc[:, d, :], in_=xT[d * 128:(d + 1) * 128, lo:lo + CH]
                    )
                own0 = 0
                kv_t0 = 1
                # previous chunk's last 128 KV tokens become this chunk's halo
                nc.vector.tensor_copy(KT[:, :, 0:HALO], KT[:, :, CH:CH + HALO])
                nc.vector.tensor_copy(V[:, 0, :], V[:, 2, :])

            kv_cols = KVW - kv_t0 * 128   # 384 (c=0) or 256

            # ---- K^T projection ----------------------------------------
            for e in range(DT):
                ps = psp.tile([128, 512], F32, tag="proj")
                for d in range(DT):
                    nc.tensor.matmul(
                        ps[:, 0:kv_cols],
                        wk_sb[d][:, e * 128:(e + 1) * 128],
                        xc[:, d, 0:kv_cols],
                        start=(d == 0),
                        stop=(d == DT - 1),
                    )
                nc.vector.tensor_scalar_add(
                    KT[:, e, kv_t0 * 128:KVW], ps[:, 0:kv_cols], bk_sb[:, e:e + 1]
                )

            # ---- V projection ------------------------------------------
            # d-outer / eh-inner: consecutive matmuls share the stationary
            # xc[:, d] tile, halving distinct weight loads.
            for t in range(kv_t0, KVW // 128):
                xcol = (t - kv_t0) * 128
                ps0 = psp.tile([128, 512], F32, tag="proj")
                ps1 = psp.tile([128, 512], F32, tag="proj")
                for d in range(DT):
                    for eh, ps in ((0, ps0), (1, ps1)):
                        nc.tensor.matmul(
                            ps,
                            xc[:, d, xcol:xcol + 128],
                            wv_sb[d][:, eh * 512:(eh + 1) * 512],
                            start=(d == 0),
                            stop=(d == DT - 1),
                        )
                for eh, ps in ((0, ps0), (1, ps1)):
                    nc.vector.tensor_add(
                        V[:, t, eh * 512:(eh + 1) * 512],
                        ps,
                        bv_sb[:, eh * 512:(eh + 1) * 512],
                    )

            # ---- Q^T projection ----------------------------------------
            for e in range(DT):
                ps = psp.tile([128, 512], F32, tag="proj")
                for d in range(DT):
                    nc.tensor.matmul(
                        ps[:, 0:CH],
                        wq_sb[d][:, e * 128:(e + 1) * 128],
                        xc[:, d, own0:own0 + CH],
                        start=(d == 0),
                        stop=(d == DT - 1),
                    )
                nc.vector.tensor_scalar_add(
                    QT[:, e, :], ps[:, 0:CH], bq_sb[:, e:e + 1]
                )

            # ---- attention for the chunk's block pair ------------------
            # sT[k, r*CH + q] = sum_e KT[e, r*128+k] * QT[e, q]
            ps_s = pss.tile([128, 3 * CH], F32, tag="sT")
            for r in range(3):
                for e in range(DT):
                    nc.tensor.matmul(
                        ps_s[:, r * CH:(r + 1) * CH],
                        KT[:, e, r * 128:(r + 1) * 128],
                        QT[:, e, :],
                        start=(e == 0),
                        stop=(e == DT - 1),
                    )
            # wT = exp(sT * scale + mask)
            m_sb = maskf_sb if c == 0 else mask_sb
            stt = work.tile([128, 3 * CH], F32, tag="stt")
            nc.vector.scalar_tensor_tensor(
                stt, ps_s, SCALE, m_sb,
                mybir.AluOpType.mult, mybir.AluOpType.add,
            )
            wT = work.tile([128, 3 * CH], F32R, tag="wT")
            nc.scalar.activation(wT, stt, mybir.ActivationFunctionType.Exp)

            # o[q, e] (+ denominator in col 1024) per 128-query half
            for half in range(2):
                r_lo = half          # half A uses r in {0,1}, half B {1,2}
                ps_o = pso.tile([128, 1032], F32, tag="o")
                for ri, r in enumerate((r_lo, r_lo + 1)):
                    lhsT = wT[:, r * CH + half * 128: r * CH + half * 128 + 128]
                    for eh in range(2):
                        nc.tensor.matmul(
                            ps_o[:, eh * 512:(eh + 1) * 512],
                            lhsT,
                            V[:, r, eh * 512:(eh + 1) * 512],
                            start=(ri == 0),
                            stop=(ri == 1),
                        )
                    nc.tensor.matmul(
                        ps_o[:, 1024:1032],
                        lhsT,
                        ones_sb,
                        start=(ri == 0),
                        stop=(ri == 1),
                    )
                recip = outp.tile([128, 1], F32, tag="recip")
                nc.vector.reciprocal(recip, ps_o[:, 1024:1025])
                o_sb = outp.tile([128, D], F32, tag="o_sb")
                nc.vector.tensor_scalar_mul(
                    o_sb[:, 0:512], ps_o[:, 0:512], recip
                )
                nc.vector.tensor_scalar_mul(
                    o_sb[:, 512:1024], ps_o[:, 512:1024], recip
                )
                row0 = c * CH + half * 128
                nc.sync.dma_start(out=out[row0:row0 + 128, :], in_=o_sb)

    _split_multi_waits(nc)
    return nc


_GRAPH = None


def _get_graph():
    global _GRAPH
    if _GRAPH is None:
        _GRAPH = _build_graph()
    return _GRAPH


# ---------------------------------------------------------------------------
def _make_masks():
    """Additive masks in [k_partition, r*CH + q_free] layout (post-scale)."""
    kp = np.arange(128)[:, None]
    qf = np.arange(CH)[None, :]
    m0 = np.where(kp > qf, 0.0, NEG)                      # r=0 (prev block)
    m1 = np.where((kp <= qf) & (kp > qf - 128), 0.0, NEG)  # r=1
    m2 = np.where(kp <= qf - 128, 0.0, NEG)                # r=2
    m = np.concatenate([m0, m1, m2], axis=1).astype(np.float32)
    mf = m.copy()
    mf[:, 0:CH] = NEG   # global block 0: the halo "previous block" is padding
    return m, mf


def kernel(x, Wq, bq, Wk, bk, Wv, bv, span):
    x = np.asarray(x)
    span_i = int(np.asarray(span))
    assert span_i == SPAN, f"kernel hardcodes span={SPAN}, got {span_i}"
    assert x.shape == (B, T, D)

    nc = _get_graph()

    wqT = np.ascontiguousarray(np.asarray(Wq).T).astype(np.float32, copy=False)
    wkT = np.ascontiguousarray(np.asarray(Wk).T).astype(np.float32, copy=False)
    wvT = np.ascontiguousarray(np.asarray(Wv).T).astype(np.float32, copy=False)
    bq8 = np.ascontiguousarray(np.asarray(bq).reshape(DT, 128).T).astype(np.float32, copy=False)
    bk8 = np.ascontiguousarray(np.asarray(bk).reshape(DT, 128).T).astype(np.float32, copy=False)
    bvb = np.ascontiguousarray(np.broadcast_to(np.asarray(bv), (128, D))).astype(np.float32, copy=False)
    m, mf = _make_masks()
    ones = np.ones((128, 8), np.float32)

    in_maps = []
    for core in range(NCORES):
        b, h = divmod(core, 2)
        lo = h * TOKQ - HALO
        hi = (h + 1) * TOKQ
        xs = np.zeros((HALO + TOKQ, D), np.float32)
        if lo < 0:
            xs[HALO:] = x[b, 0:hi]
        else:
            xs[:] = x[b, lo:hi]
        xT = np.ascontiguousarray(xs.T)
        in_maps.append({
            "xT": xT, "wqT": wqT, "wkT": wkT, "wvT": wvT,
            "bq8": bq8, "bk8": bk8, "bvb": bvb,
            "maskc": m, "maskf": (mf if h == 0 else m), "onesp": ones,
        })

    res = run_bass_kernel_spmd(nc, in_maps, core_ids=list(range(NCORES)))

    out = np.empty((B, T, D), np.float32)
    for core in range(NCORES):
        b, h = divmod(core, 2)
        out[b, h * TOKQ:(h + 1) * TOKQ] = res.results[core]["out"]
    return out


# revision 11
# speedup vs baseline: 1.0603x; 1.0603x over previous
"""Trainium2 kernel for AdaptiveAttention (QKV projection + causal
sliding-window attention, span=128) on 8 NeuronCores.

Sharding: sequence-parallel with a 1-block halo — 8 shards of
(batch b, sequence half h): each core owns 2048 query tokens and receives
128 halo tokens of x so it can compute the previous block's K/V locally.
No collectives are needed (window attention is local).

Per-core layout strategy (all matmuls in float32r, full PE speed at N>=256):
  - x is passed pre-transposed (d-major) so Q^T/K^T projections and the
    V projection need no on-chip transposes:
      Q^T[e,t] = sum_d WqT[d,e] * xT[d,t]   (lhsT=WqT tile, rhs=xT)
      V[t,e]   = sum_d xT[d,t]  * WvT[d,e]  (lhsT=xT tile,  rhs=WvT)
  - scores are computed transposed, sT[k,q] = sum_e KT[e,k]*QT[e,q],
    masked + scaled on DVE, exponentiated on ACT (no max subtraction:
    scores are O(+-8) so exp is safe in fp32), then
      o[q,e] = sum_k wT[k,q] * V[k,e]
    with the softmax denominator obtained from an extra ones-column
    matmul into the same PSUM tile; final normalize is a per-partition
    scalar multiply fused with the PSUM->SBUF copy.
"""

import sys
import types

if "/opt/trn_rl_repo" not in sys.path:
    sys.path.insert(0, "/opt/trn_rl_repo")

import numpy as np
from contextlib import ExitStack

import concourse.bass as bass
import concourse.mybir as mybir
import concourse.tile as tile
from concourse.bass_utils import run_bass_kernel_spmd
from concourse.vector_clock import ScopedClock

# ---------------------------------------------------------------------------
# Problem constants (hardcoded per spec)
B, T, D = 4, 4096, 1024
SPAN = 128
NCORES = 8
TOKQ = T // 2           # 2048 query tokens per core
HALO = SPAN             # 128
CH = 256                # query chunk size (2 blocks = 1 block-pair)
NCH = TOKQ // CH        # 8 chunks
KVW = CH + HALO         # 384 KV tokens visible per chunk
DT = D // 128           # 8 d-tiles
NEG = -1.0e9
SCALE = 1.0 / 32.0      # 1/sqrt(D)

F32 = mybir.dt.float32
F32R = mybir.dt.float32r

# ---------------------------------------------------------------------------
# Walrus in this toolchain caps semaphore waits per instruction; Tile's
# kernel-tail Drain can exceed it. Chunk excess waits onto extra drains.
_MAX_WAITS = 1


def _patched_drain_and_barrier(self, tick_clock, wait_clock):
    nc = self.nc
    drain_inst = nc.sync.drain()
    wait_clock.add_sem_waits(
        drain_inst.ins, ScopedClock({None: tick_clock.global_clock})
    )
    si = drain_inst.ins.sync_info
    if si is not None and len(si.on_wait) > _MAX_WAITS:
        waits = list(si.on_wait)
        si.on_wait[:] = waits[:_MAX_WAITS]
        rest = waits[_MAX_WAITS:]
        while rest:
            extra = nc.sync.drain(fusable=False)
            extra.ins.sync_info = mybir.SyncInfo(
                on_wait=rest[:_MAX_WAITS], on_update=[]
            )
            rest = rest[_MAX_WAITS:]
    nc.all_engine_barrier()
    assert self.sems is not None
    popped = nc._tile_sem_poison_stack.pop()
    assert popped is self._sem_poison
    nc.clear_and_free_semaphores(list(self.sems.allocated().values()))
    nc.all_engine_barrier()


def _install_drain_patch():
    if getattr(tile.TileContext, "_drain_patch_installed", False):
        return
    tile.TileContext._drain_and_barrier = _patched_drain_and_barrier
    tile.TileContext._drain_patch_installed = True


def _split_multi_waits(nc, max_waits=_MAX_WAITS):
    """Walrus here supports one semaphore wait per instruction; hoist excess
    waits onto same-engine NoOps inserted immediately before."""
    for fn in nc.m.functions:
        for bb in fn.blocks:
            insts = bb.instructions
            out = []
            changed = False
            for inst in insts:
                si = getattr(inst, "sync_info", None)
                waits = list(si.on_wait) if si is not None else []
                if len(waits) > max_waits:
                    changed = True
                    for w in waits[:-max_waits]:
                        out.append(mybir.InstNoOp(
                            name=nc.get_next_instruction_name(),
                            sync_info=mybir.SyncInfo(on_wait=[w], on_update=[]),
                            bass_nofuse=True,
                            engine=inst.engine,
                        ))
                    si.on_wait[:] = waits[-max_waits:]
                out.append(inst)
            if changed:
                bb.instructions = out


# ---------------------------------------------------------------------------
def _build_graph():
    """Build the per-core Bass graph (SPMD: identical on all 8 cores)."""
    _install_drain_patch()
    nc = bass.Bass()

    # DRAM parameters. Matmul operands are declared float32r (same bits as
    # f32; the PE rounds to its reduced internal format).
    xT = nc.declare_dram_parameter("xT", [D, HALO + TOKQ], F32R, isOutput=False)
    wqT = nc.declare_dram_parameter("wqT", [D, D], F32R, isOutput=False)
    wkT = nc.declare_dram_parameter("wkT", [D, D], F32R, isOutput=False)
    wvT = nc.declare_dram_parameter("wvT", [D, D], F32R, isOutput=False)
    bq8 = nc.declare_dram_parameter("bq8", [128, DT], F32, isOutput=False)
    bk8 = nc.declare_dram_parameter("bk8", [128, DT], F32, isOutput=False)
    bvb = nc.declare_dram_parameter("bvb", [128, D], F32, isOutput=False)
    maskc = nc.declare_dram_parameter("maskc", [128, 3 * CH], F32, isOutput=False)
    maskf = nc.declare_dram_parameter("maskf", [128, 3 * CH], F32, isOutput=False)
    onesp = nc.declare_dram_parameter("onesp", [128, 8], F32R, isOutput=False)
    out = nc.declare_dram_parameter("out", [TOKQ, D], F32, isOutput=True)

    with ExitStack() as ctx:
        tc = ctx.enter_context(tile.TileContext(nc))
        consts = ctx.enter_context(tc.tile_pool(name="consts", bufs=1))
        persist = ctx.enter_context(tc.tile_pool(name="persist", bufs=1))
        xc0p = ctx.enter_context(tc.tile_pool(name="xc0p", bufs=1))
        xcp = ctx.enter_context(tc.tile_pool(name="xcp", bufs=2))
        work = ctx.enter_context(tc.tile_pool(name="work", bufs=1))
        outp = ctx.enter_context(tc.tile_pool(name="outp", bufs=2))
        psp = ctx.enter_context(tc.tile_pool(name="psp", bufs=2, space="PSUM"))
        pss = ctx.enter_context(tc.tile_pool(name="pss", bufs=1, space="PSUM"))
        pso = ctx.enter_context(tc.tile_pool(name="pso", bufs=1, space="PSUM"))

        # --- resident weights + constants -------------------------------
        # DMA order matters: the K projection of chunk 0 needs xc0 + Wk, so
        # issue those first to start the PE as early as possible; Wq next
        # (Q proj), Wv last (V proj runs after K proj in chunk 0).
        xc0 = xc0p.tile([128, DT, KVW], F32R, tag="xc0")
        wk_sb = []
        for d in range(DT):
            nc.sync.dma_start(
                out=xc0[:, d, :], in_=xT[d * 128:(d + 1) * 128, 0:KVW]
            )
            wk = consts.tile([128, D], F32R, tag=f"wk{d}")
            nc.sync.dma_start(out=wk, in_=wkT[d * 128:(d + 1) * 128, :])
            wk_sb.append(wk)
        wq_sb, wv_sb = [], []
        for d in range(DT):
            wv = consts.tile([128, D], F32R, tag=f"wv{d}")
            nc.sync.dma_start(out=wv, in_=wvT[d * 128:(d + 1) * 128, :])
            wv_sb.append(wv)
        for d in range(DT):
            wq = consts.tile([128, D], F32R, tag=f"wq{d}")
            nc.sync.dma_start(out=wq, in_=wqT[d * 128:(d + 1) * 128, :])
            wq_sb.append(wq)

        bq_sb = consts.tile([128, DT], F32, tag="bq")
        nc.sync.dma_start(out=bq_sb, in_=bq8[:, :])
        bk_sb = consts.tile([128, DT], F32, tag="bk")
        nc.sync.dma_start(out=bk_sb, in_=bk8[:, :])
        bv_sb = consts.tile([128, D], F32, tag="bv")
        nc.sync.dma_start(out=bv_sb, in_=bvb[:, :])
        mask_sb = consts.tile([128, 3 * CH], F32, tag="mask")
        nc.sync.dma_start(out=mask_sb, in_=maskc[:, :])
        maskf_sb = consts.tile([128, 3 * CH], F32, tag="maskf")
        nc.sync.dma_start(out=maskf_sb, in_=maskf[:, :])
        ones_sb = consts.tile([128, 8], F32R, tag="ones")
        nc.sync.dma_start(out=ones_sb, in_=onesp[:, :])

        # --- persistent per-chunk state ---------------------------------
        # KT: K^T, e-tile-major [128, e_tile, kv_col], kv_col in [0, 384):
        #   kv token (chunk-local) = chunk_start - 128 + kv_col
        # V: token-tile-major [128, tok_tile(3), e]
        # QT: Q^T [128, e_tile, q_col], q_col in [0, 256)
        KT = persist.tile([128, DT, KVW], F32R, tag="KT")
        V = persist.tile([128, KVW // 128, D], F32R, tag="V")
        QT = persist.tile([128, DT, CH], F32R, tag="QT")

        for c in range(NCH):
            # ---- x chunk DMA (d-major) ---------------------------------
            if c == 0:
                # xc0 (incl. halo: xT cols [0, 384)) was DMA'd up front
                xc = xc0
                own0 = HALO          # xc col of first own token
                kv_t0 = 0            # first KV token-tile to project
            else:
                xc = xcp.tile([128, DT, CH], F32R, tag="xc")
                lo = HALO + c * CH
                for d in range(DT):
                    nc.sync.dma_start(
                        out=xc[:, d, :], in_=xT[d * 128:(d + 1) * 128, lo:lo + CH]
                    )
                own0 = 0
                kv_t0 = 1
                # previous chunk's last 128 KV tokens become this chunk's halo
                nc.vector.tensor_copy(KT[:, :, 0:HALO], KT[:, :, CH:CH + HALO])
                nc.vector.tensor_copy(V[:, 0, :], V[:, 2, :])

            kv_cols = KVW - kv_t0 * 128   # 384 (c=0) or 256

            # ---- K^T projection ----------------------------------------
            for e in range(DT):
                ps = psp.tile([128, 512], F32, tag="proj")
                for d in range(DT):
                    nc.tensor.matmul(
                        ps[:, 0:kv_cols],
                        wk_sb[d][:, e * 128:(e + 1) * 128],
                        xc[:, d, 0:kv_cols],
                        start=(d == 0),
                        stop=(d == DT - 1),
                    )
                nc.vector.tensor_scalar_add(
                    KT[:, e, kv_t0 * 128:KVW], ps[:, 0:kv_cols], bk_sb[:, e:e + 1]
                )

            # ---- V projection ------------------------------------------
            for t in range(kv_t0, KVW // 128):
                xcol = (t - kv_t0) * 128
                for eh in range(2):
                    ps = psp.tile([128, 512], F32, tag="proj")
                    for d in range(DT):
                        nc.tensor.matmul(
                            ps,
                            xc[:, d, xcol:xcol + 128],
                            wv_sb[d][:, eh * 512:(eh + 1) * 512],
                            start=(d == 0),
                            stop=(d == DT - 1),
                        )
                    nc.vector.tensor_add(
                        V[:, t, eh * 512:(eh + 1) * 512],
                        ps,
                        bv_sb[:, eh * 512:(eh + 1) * 512],
                    )

            # ---- Q^T projection ----------------------------------------
            for e in range(DT):
                ps = psp.tile([128, 512], F32, tag="proj")
                for d in range(DT):
                    nc.tensor.matmul(
                        ps[:, 0:CH],
                        wq_sb[d][:, e * 128:(e + 1) * 128],
                        xc[:, d, own0:own0 + CH],
                        start=(d == 0),
                        stop=(d == DT - 1),
                    )
                nc.vector.tensor_scalar_add(
                    QT[:, e, :], ps[:, 0:CH], bq_sb[:, e:e + 1]
                )

            # ---- attention for the chunk's block pair ------------------
            # sT[k, r*CH + q] = sum_e KT[e, r*128+k] * QT[e, q]
            ps_s = pss.tile([128, 3 * CH], F32, tag="sT")
            for r in range(3):
                for e in range(DT):
                    nc.tensor.matmul(
                        ps_s[:, r * CH:(r + 1) * CH],
                        KT[:, e, r * 128:(r + 1) * 128],
                        QT[:, e, :],
                        start=(e == 0),
                        stop=(e == DT - 1),
                    )
            # wT = exp(sT * scale + mask)
            m_sb = maskf_sb if c == 0 else mask_sb
            stt = work.tile([128, 3 * CH], F32, tag="stt")
            nc.vector.scalar_tensor_tensor(
                stt, ps_s, SCALE, m_sb,
                mybir.AluOpType.mult, mybir.AluOpType.add,
            )
            wT = work.tile([128, 3 * CH], F32R, tag="wT")
            nc.scalar.activation(wT, stt, mybir.ActivationFunctionType.Exp)

            # o[q, e] (+ denominator in col 1024) per 128-query half
            for half in range(2):
                r_lo = half          # half A uses r in {0,1}, half B {1,2}
                ps_o = pso.tile([128, 1032], F32, tag="o")
                for ri, r in enumerate((r_lo, r_lo + 1)):
                    lhsT = wT[:, r * CH + half * 128: r * CH + half * 128 + 128]
                    for eh in range(2):
                        nc.tensor.matmul(
                            ps_o[:, eh * 512:(eh + 1) * 512],
                            lhsT,
                            V[:, r, eh * 512:(eh + 1) * 512],
                            start=(ri == 0),
                            stop=(ri == 1),
                        )
                    nc.tensor.matmul(
                        ps_o[:, 1024:1032],
                        lhsT,
                        ones_sb,
                        start=(ri == 0),
                        stop=(ri == 1),
                    )
                recip = outp.tile([128, 1], F32, tag="recip")
                nc.vector.reciprocal(recip, ps_o[:, 1024:1025])
                o_sb = outp.tile([128, D], F32, tag="o_sb")
                nc.vector.tensor_scalar_mul(
                    o_sb[:, 0:512], ps_o[:, 0:512], recip
                )
                nc.vector.tensor_scalar_mul(
                    o_sb[:, 512:1024], ps_o[:, 512:1024], recip
                )
                row0 = c * CH + half * 128
                nc.sync.dma_start(out=out[row0:row0 + 128, :], in_=o_sb)

    _split_multi_waits(nc)
    return nc


_GRAPH = None


def _get_graph():
    global _GRAPH
    if _GRAPH is None:
        _GRAPH = _build_graph()
    return _GRAPH


# ---------------------------------------------------------------------------
def _make_masks():
    """Additive masks in [k_partition, r*CH + q_free] layout (post-scale)."""
    kp = np.arange(128)[:, None]
    qf = np.arange(CH)[None, :]
    m0 = np.where(kp > qf, 0.0, NEG)                      # r=0 (prev block)
    m1 = np.where((kp <= qf) & (kp > qf - 128), 0.0, NEG)  # r=1
    m2 = np.where(kp <= qf - 128, 0.0, NEG)                # r=2
    m = np.concatenate([m0, m1, m2], axis=1).astype(np.float32)
    mf = m.copy()
    mf[:, 0:CH] = NEG   # global block 0: the halo "previous block" is padding
    return m, mf


def kernel(x, Wq, bq, Wk, bk, Wv, bv, span):
    x = np.asarray(x)
    span_i = int(np.asarray(span))
    assert span_i == SPAN, f"kernel hardcodes span={SPAN}, got {span_i}"
    assert x.shape == (B, T, D)

    nc = _get_graph()

    wqT = np.ascontiguousarray(np.asarray(Wq).T).astype(np.float32, copy=False)
    wkT = np.ascontiguousarray(np.asarray(Wk).T).astype(np.float32, copy=False)
    wvT = np.ascontiguousarray(np.asarray(Wv).T).astype(np.float32, copy=False)
    bq8 = np.ascontiguousarray(np.asarray(bq).reshape(DT, 128).T).astype(np.float32, copy=False)
    bk8 = np.ascontiguousarray(np.asarray(bk).reshape(DT, 128).T).astype(np.float32, copy=False)
    bvb = np.ascontiguousarray(np.broadcast_to(np.asarray(bv), (128, D))).astype(np.float32, copy=False)
    m, mf = _make_masks()
    ones = np.ones((128, 8), np.float32)

    in_maps = []
    for core in range(NCORES):
        b, h = divmod(core, 2)
        lo = h * TOKQ - HALO
        hi = (h + 1) * TOKQ
        xs = np.zeros((HALO + TOKQ, D), np.float32)
        if lo < 0:
            xs[HALO:] = x[b, 0:hi]
        else:
            xs[:] = x[b, lo:hi]
        xT = np.ascontiguousarray(xs.T)
        in_maps.append({
            "xT": xT, "wqT": wqT, "wkT": wkT, "wvT": wvT,
            "bq8": bq8, "bk8": bk8, "bvb": bvb,
            "maskc": m, "maskf": (mf if h == 0 else m), "onesp": ones,
        })

    res = run_bass_kernel_spmd(nc, in_maps, core_ids=list(range(NCORES)))

    out = np.empty((B, T, D), np.float32)
    for core in range(NCORES):
        b, h = divmod(core, 2)
        out[b, h * TOKQ:(h + 1) * TOKQ] = res.results[core]["out"]
    return out


# revision 12
# speedup vs baseline: 1.1689x; 1.1024x over previous
"""Trainium2 kernel for AdaptiveAttention (QKV projection + causal
sliding-window attention, span=128) on 8 NeuronCores.

Sharding: sequence-parallel with a 1-block halo — 8 shards of
(batch b, sequence half h): each core owns 2048 query tokens and receives
128 halo tokens of x so it can compute the previous block's K/V locally.
No collectives are needed (window attention is local).

Per-core layout strategy (all matmuls in float32r, full PE speed at N>=256):
  - x is passed pre-transposed (d-major) so Q^T/K^T projections and the
    V projection need no on-chip transposes:
      Q^T[e,t] = sum_d WqT[d,e] * xT[d,t]   (lhsT=WqT tile, rhs=xT)
      V[t,e]   = sum_d xT[d,t]  * WvT[d,e]  (lhsT=xT tile,  rhs=WvT)
  - scores are computed transposed, sT[k,q] = sum_e KT[e,k]*QT[e,q],
    masked + scaled on DVE, exponentiated on ACT (no max subtraction:
    scores are O(+-8) so exp is safe in fp32), then
      o[q,e] = sum_k wT[k,q] * V[k,e]
    with the softmax denominator obtained from an extra ones-column
    matmul into the same PSUM tile; final normalize is a per-partition
    scalar multiply fused with the PSUM->SBUF copy.
"""

import sys
import types

if "/opt/trn_rl_repo" not in sys.path:
    sys.path.insert(0, "/opt/trn_rl_repo")

import numpy as np
from contextlib import ExitStack

import concourse.bass as bass
import concourse.mybir as mybir
import concourse.tile as tile
from concourse.bass_utils import run_bass_kernel_spmd
from concourse.vector_clock import ScopedClock

# ---------------------------------------------------------------------------
# Problem constants (hardcoded per spec)
B, T, D = 4, 4096, 1024
SPAN = 128
NCORES = 8
TOKQ = T // 2           # 2048 query tokens per core
HALO = SPAN             # 128
CH = 256                # query chunk size (2 blocks = 1 block-pair)
NCH = TOKQ // CH        # 8 chunks
KVW = CH + HALO         # 384 KV tokens visible per chunk
DT = D // 128           # 8 d-tiles
NEG = -1.0e9
SCALE = 1.0 / 32.0      # 1/sqrt(D)

F32 = mybir.dt.float32
F32R = mybir.dt.float32r
F16 = mybir.dt.float16

# ---------------------------------------------------------------------------
# Walrus in this toolchain caps semaphore waits per instruction; Tile's
# kernel-tail Drain can exceed it. Chunk excess waits onto extra drains.
_MAX_WAITS = 1


def _patched_drain_and_barrier(self, tick_clock, wait_clock):
    nc = self.nc
    drain_inst = nc.sync.drain()
    wait_clock.add_sem_waits(
        drain_inst.ins, ScopedClock({None: tick_clock.global_clock})
    )
    si = drain_inst.ins.sync_info
    if si is not None and len(si.on_wait) > _MAX_WAITS:
        waits = list(si.on_wait)
        si.on_wait[:] = waits[:_MAX_WAITS]
        rest = waits[_MAX_WAITS:]
        while rest:
            extra = nc.sync.drain(fusable=False)
            extra.ins.sync_info = mybir.SyncInfo(
                on_wait=rest[:_MAX_WAITS], on_update=[]
            )
            rest = rest[_MAX_WAITS:]
    nc.all_engine_barrier()
    assert self.sems is not None
    popped = nc._tile_sem_poison_stack.pop()
    assert popped is self._sem_poison
    nc.clear_and_free_semaphores(list(self.sems.allocated().values()))
    nc.all_engine_barrier()


def _install_drain_patch():
    if getattr(tile.TileContext, "_drain_patch_installed", False):
        return
    tile.TileContext._drain_and_barrier = _patched_drain_and_barrier
    tile.TileContext._drain_patch_installed = True


def _split_multi_waits(nc, max_waits=_MAX_WAITS):
    """Walrus here supports one semaphore wait per instruction; hoist excess
    waits onto same-engine NoOps inserted immediately before."""
    for fn in nc.m.functions:
        for bb in fn.blocks:
            insts = bb.instructions
            out = []
            changed = False
            for inst in insts:
                si = getattr(inst, "sync_info", None)
                waits = list(si.on_wait) if si is not None else []
                if len(waits) > max_waits:
                    changed = True
                    for w in waits[:-max_waits]:
                        out.append(mybir.InstNoOp(
                            name=nc.get_next_instruction_name(),
                            sync_info=mybir.SyncInfo(on_wait=[w], on_update=[]),
                            bass_nofuse=True,
                            engine=inst.engine,
                        ))
                    si.on_wait[:] = waits[-max_waits:]
                out.append(inst)
            if changed:
                bb.instructions = out


# ---------------------------------------------------------------------------
def _build_graph():
    """Build the per-core Bass graph (SPMD: identical on all 8 cores)."""
    _install_drain_patch()
    nc = bass.Bass()

    # DRAM parameters. Matmul operands are declared float32r (same bits as
    # f32; the PE rounds to its reduced internal format).
    xT = nc.declare_dram_parameter("xT", [D, HALO + TOKQ], F16, isOutput=False)
    wqT = nc.declare_dram_parameter("wqT", [D, D], F16, isOutput=False)
    wkT = nc.declare_dram_parameter("wkT", [D, D], F16, isOutput=False)
    wvT = nc.declare_dram_parameter("wvT", [D, D], F16, isOutput=False)
    bq8 = nc.declare_dram_parameter("bq8", [128, DT], F32, isOutput=False)
    bk8 = nc.declare_dram_parameter("bk8", [128, DT], F32, isOutput=False)
    bvb = nc.declare_dram_parameter("bvb", [128, D], F32, isOutput=False)
    maskc = nc.declare_dram_parameter("maskc", [128, 3 * CH], F32, isOutput=False)
    maskf = nc.declare_dram_parameter("maskf", [128, 3 * CH], F32, isOutput=False)
    onesp = nc.declare_dram_parameter("onesp", [128, 8], F32R, isOutput=False)
    out = nc.declare_dram_parameter("out", [TOKQ, D], F32, isOutput=True)

    with ExitStack() as ctx:
        tc = ctx.enter_context(tile.TileContext(nc))
        consts = ctx.enter_context(tc.tile_pool(name="consts", bufs=1))
        persist = ctx.enter_context(tc.tile_pool(name="persist", bufs=1))
        xc0p = ctx.enter_context(tc.tile_pool(name="xc0p", bufs=1))
        xcp = ctx.enter_context(tc.tile_pool(name="xcp", bufs=2))
        work = ctx.enter_context(tc.tile_pool(name="work", bufs=1))
        outp = ctx.enter_context(tc.tile_pool(name="outp", bufs=2))
        psp = ctx.enter_context(tc.tile_pool(name="psp", bufs=2, space="PSUM"))
        pss = ctx.enter_context(tc.tile_pool(name="pss", bufs=1, space="PSUM"))
        pso = ctx.enter_context(tc.tile_pool(name="pso", bufs=1, space="PSUM"))

        # --- resident weights + constants -------------------------------
        # DMA order matters: the K projection of chunk 0 needs xc0 + Wk, so
        # issue those first to start the PE as early as possible; Wq next
        # (Q proj), Wv last (V proj runs after K proj in chunk 0).
        xc0 = xc0p.tile([128, DT, KVW], F16, tag="xc0")
        wk_sb = []
        for d in range(DT):
            nc.sync.dma_start(
                out=xc0[:, d, :], in_=xT[d * 128:(d + 1) * 128, 0:KVW]
            )
            wk = consts.tile([128, D], F16, tag=f"wk{d}")
            nc.sync.dma_start(out=wk, in_=wkT[d * 128:(d + 1) * 128, :])
            wk_sb.append(wk)
        wq_sb, wv_sb = [], []
        for d in range(DT):
            wv = consts.tile([128, D], F16, tag=f"wv{d}")
            nc.sync.dma_start(out=wv, in_=wvT[d * 128:(d + 1) * 128, :])
            wv_sb.append(wv)
        for d in range(DT):
            wq = consts.tile([128, D], F16, tag=f"wq{d}")
            nc.sync.dma_start(out=wq, in_=wqT[d * 128:(d + 1) * 128, :])
            wq_sb.append(wq)

        bq_sb = consts.tile([128, DT], F32, tag="bq")
        nc.sync.dma_start(out=bq_sb, in_=bq8[:, :])
        bk_sb = consts.tile([128, DT], F32, tag="bk")
        nc.sync.dma_start(out=bk_sb, in_=bk8[:, :])
        bv_sb = consts.tile([128, D], F32, tag="bv")
        nc.sync.dma_start(out=bv_sb, in_=bvb[:, :])
        mask_sb = consts.tile([128, 3 * CH], F32, tag="mask")
        nc.sync.dma_start(out=mask_sb, in_=maskc[:, :])
        maskf_sb = consts.tile([128, 3 * CH], F32, tag="maskf")
        nc.sync.dma_start(out=maskf_sb, in_=maskf[:, :])
        ones_sb = consts.tile([128, 8], F32R, tag="ones")
        nc.sync.dma_start(out=ones_sb, in_=onesp[:, :])

        # --- persistent per-chunk state ---------------------------------
        # KT: K^T, e-tile-major [128, e_tile, kv_col], kv_col in [0, 384):
        #   kv token (chunk-local) = chunk_start - 128 + kv_col
        # V: token-tile-major [128, tok_tile(3), e]
        # QT: Q^T [128, e_tile, q_col], q_col in [0, 256)
        KT = persist.tile([128, DT, KVW], F32R, tag="KT")
        V = persist.tile([128, KVW // 128, D], F32R, tag="V")
        QT = persist.tile([128, DT, CH], F32R, tag="QT")

        for c in range(NCH):
            # ---- x chunk DMA (d-major) ---------------------------------
            if c == 0:
                # xc0 (incl. halo: xT cols [0, 384)) was DMA'd up front
                xc = xc0
                own0 = HALO          # xc col of first own token
                kv_t0 = 0            # first KV token-tile to project
            else:
                xc = xcp.tile([128, DT, CH], F16, tag="xc")
                lo = HALO + c * CH
                for d in range(DT):
                    nc.sync.dma_start(
                        out=xc[:, d, :], in_=xT[d * 128:(d + 1) * 128, lo:lo + CH]
                    )
                own0 = 0
                kv_t0 = 1
                # previous chunk's last 128 KV tokens become this chunk's halo
                nc.vector.tensor_copy(KT[:, :, 0:HALO], KT[:, :, CH:CH + HALO])
                nc.vector.tensor_copy(V[:, 0, :], V[:, 2, :])

            kv_cols = KVW - kv_t0 * 128   # 384 (c=0) or 256

            # ---- K^T projection ----------------------------------------
            for e in range(DT):
                ps = psp.tile([128, 512], F32, tag="proj")
                for d in range(DT):
                    nc.tensor.matmul(
                        ps[:, 0:kv_cols],
                        wk_sb[d][:, e * 128:(e + 1) * 128],
                        xc[:, d, 0:kv_cols],
                        start=(d == 0),
                        stop=(d == DT - 1),
                    )
                nc.vector.tensor_scalar_add(
                    KT[:, e, kv_t0 * 128:KVW], ps[:, 0:kv_cols], bk_sb[:, e:e + 1]
                )

            # ---- V projection ------------------------------------------
            for t in range(kv_t0, KVW // 128):
                xcol = (t - kv_t0) * 128
                for eh in range(2):
                    ps = psp.tile([128, 512], F32, tag="proj")
                    for d in range(DT):
                        nc.tensor.matmul(
                            ps,
                            xc[:, d, xcol:xcol + 128],
                            wv_sb[d][:, eh * 512:(eh + 1) * 512],
                            start=(d == 0),
                            stop=(d == DT - 1),
                        )
                    nc.vector.tensor_add(
                        V[:, t, eh * 512:(eh + 1) * 512],
                        ps,
                        bv_sb[:, eh * 512:(eh + 1) * 512],
                    )

            # ---- Q^T projection ----------------------------------------
            for e in range(DT):
                ps = psp.tile([128, 512], F32, tag="proj")
                for d in range(DT):
                    nc.tensor.matmul(
                        ps[:, 0:CH],
                        wq_sb[d][:, e * 128:(e + 1) * 128],
                        xc[:, d, own0:own0 + CH],
                        start=(d == 0),
                        stop=(d == DT - 1),
                    )
                nc.vector.tensor_scalar_add(
                    QT[:, e, :], ps[:, 0:CH], bq_sb[:, e:e + 1]
                )

            # ---- attention for the chunk's block pair ------------------
            # sT[k, r*CH + q] = sum_e KT[e, r*128+k] * QT[e, q]
            ps_s = pss.tile([128, 3 * CH], F32, tag="sT")
            for r in range(3):
                for e in range(DT):
                    nc.tensor.matmul(
                        ps_s[:, r * CH:(r + 1) * CH],
                        KT[:, e, r * 128:(r + 1) * 128],
                        QT[:, e, :],
                        start=(e == 0),
                        stop=(e == DT - 1),
                    )
            # wT = exp(sT * scale + mask)
            m_sb = maskf_sb if c == 0 else mask_sb
            stt = work.tile([128, 3 * CH], F32, tag="stt")
            nc.vector.scalar_tensor_tensor(
                stt, ps_s, SCALE, m_sb,
                mybir.AluOpType.mult, mybir.AluOpType.add,
            )
            wT = work.tile([128, 3 * CH], F32R, tag="wT")
            nc.scalar.activation(wT, stt, mybir.ActivationFunctionType.Exp)

            # o[q, e] (+ denominator in col 1024) per 128-query half
            for half in range(2):
                r_lo = half          # half A uses r in {0,1}, half B {1,2}
                ps_o = pso.tile([128, 1032], F32, tag="o")
                for ri, r in enumerate((r_lo, r_lo + 1)):
                    lhsT = wT[:, r * CH + half * 128: r * CH + half * 128 + 128]
                    for eh in range(2):
                        nc.tensor.matmul(
                            ps_o[:, eh * 512:(eh + 1) * 512],
                            lhsT,
                            V[:, r, eh * 512:(eh + 1) * 512],
                            start=(ri == 0),
                            stop=(ri == 1),
                        )
                    nc.tensor.matmul(
                        ps_o[:, 1024:1032],
                        lhsT,
                        ones_sb,
                        start=(ri == 0),
                        stop=(ri == 1),
                    )
                recip = outp.tile([128, 1], F32, tag="recip")
                nc.vector.reciprocal(recip, ps_o[:, 1024:1025])
                o_sb = outp.tile([128, D], F32, tag="o_sb")
                nc.vector.tensor_scalar_mul(
                    o_sb[:, 0:512], ps_o[:, 0:512], recip
                )
                nc.vector.tensor_scalar_mul(
                    o_sb[:, 512:1024], ps_o[:, 512:1024], recip
                )
                row0 = c * CH + half * 128
                nc.sync.dma_start(out=out[row0:row0 + 128, :], in_=o_sb)

    _split_multi_waits(nc)
    return nc


_GRAPH = None


def _get_graph():
    global _GRAPH
    if _GRAPH is None:
        _GRAPH = _build_graph()
    return _GRAPH


# ---------------------------------------------------------------------------
def _make_masks():
    """Additive masks in [k_partition, r*CH + q_free] layout (post-scale)."""
    kp = np.arange(128)[:, None]
    qf = np.arange(CH)[None, :]
    m0 = np.where(kp > qf, 0.0, NEG)                      # r=0 (prev block)
    m1 = np.where((kp <= qf) & (kp > qf - 128), 0.0, NEG)  # r=1
    m2 = np.where(kp <= qf - 128, 0.0, NEG)                # r=2
    m = np.concatenate([m0, m1, m2], axis=1).astype(np.float32)
    mf = m.copy()
    mf[:, 0:CH] = NEG   # global block 0: the halo "previous block" is padding
    return m, mf


def kernel(x, Wq, bq, Wk, bk, Wv, bv, span):
    x = np.asarray(x)
    span_i = int(np.asarray(span))
    assert span_i == SPAN, f"kernel hardcodes span={SPAN}, got {span_i}"
    assert x.shape == (B, T, D)

    nc = _get_graph()

    wqT = np.ascontiguousarray(np.asarray(Wq).T).astype(np.float16)
    wkT = np.ascontiguousarray(np.asarray(Wk).T).astype(np.float16)
    wvT = np.ascontiguousarray(np.asarray(Wv).T).astype(np.float16)
    bq8 = np.ascontiguousarray(np.asarray(bq).reshape(DT, 128).T).astype(np.float32, copy=False)
    bk8 = np.ascontiguousarray(np.asarray(bk).reshape(DT, 128).T).astype(np.float32, copy=False)
    bvb = np.ascontiguousarray(np.broadcast_to(np.asarray(bv), (128, D))).astype(np.float32, copy=False)
    m, mf = _make_masks()
    ones = np.ones((128, 8), np.float32)

    in_maps = []
    for core in range(NCORES):
        b, h = divmod(core, 2)
        lo = h * TOKQ - HALO
        hi = (h + 1) * TOKQ
        xs = np.zeros((HALO + TOKQ, D), np.float32)
        if lo < 0:
            xs[HALO:] = x[b, 0:hi]
        else:
            xs[:] = x[b, lo:hi]
        xT = np.ascontiguousarray(xs.T).astype(np.float16)
        in_maps.append({
            "xT": xT, "wqT": wqT, "wkT": wkT, "wvT": wvT,
            "bq8": bq8, "bk8": bk8, "bvb": bvb,
            "maskc": m, "maskf": (mf if h == 0 else m), "onesp": ones,
        })

    res = run_bass_kernel_spmd(nc, in_maps, core_ids=list(range(NCORES)))

    out = np.empty((B, T, D), np.float32)
    for core in range(NCORES):
        b, h = divmod(core, 2)
        out[b, h * TOKQ:(h + 1) * TOKQ] = res.results[core]["out"]
    return out
